# revision 34
# baseline (speedup 1.0000x reference)
"""Trainium2 Bass kernel for nn_EnokeeEncoder (ragged mention pooling +
4-layer transformer + 50k-entity classifier), data-parallel over batch
across 8 NeuronCores.

Layout strategy per core (2 batches, 256 mention-tokens):
  - residual stream x: token-major [128 tokens/p, 768] f32 (LN/softmax easy)
  - matmul chains run feature-major via PE transposes of x
  - all big matmuls in bf16 (weights pre-cast+pre-transposed on host),
    f32 accumulation in PSUM; LN / softmax / residual in f32.
  - attention scores computed k-major (lhsT=k, rhs=q) so exp output feeds
    the AV matmul directly as the stationary operand - no transpose.
  - AV matmuls append a ones-column per head (v_aug) giving the softmax
    denominator; all 12 heads of a batch pack into two PSUM tiles so the
    normalization is 2 reciprocals + 2 strided tensor_tensor muls.
  - classifier: f32 psum halves (512-wide, one bank each) rotated across
    all three psum pools, evicted to a 1024-wide bf16 tile, chunk-major
    DRAM layout, unpacked/cast to f32 on host; w2 chunks prefetched into
    SBUF during layers 1-3 to smooth HBM demand.
"""

import sys

for _p in ("/opt/trn_rl_repo",):
    if _p not in sys.path:
        sys.path.insert(0, _p)

import numpy as np
import ml_dtypes

BF16 = ml_dtypes.bfloat16

B, M, L, S = 16, 128, 32, 512
D, H, DFF, NL = 768, 12, 3072, 4
NE = 50000
HD = D // H
EPS = 1e-5
N_CORES = 8
BL = B // N_CORES          # batches per core
P = 128
KD = D // P                # 6 k-tiles over D
KF = DFF // P              # 24 k-tiles over DFF
MT = BL                    # token m-tiles per core (M == P)
NQK = 2 * D // P           # 12 m-tiles over q,k features
CCH = 1024                 # classifier chunk (bf16 psum bank)
NCH = (NE + CCH - 1) // CCH  # 49
W2_PREFETCH = 17

KERNEL_DEBUG = False
_CACHE = {}


def _build(cfg):
    (attn_b_val, qkb_nz, vb_nz, outb_nz, f1b_nz, ff2b_nz, ln1_nt, ln2_nt,
     debug) = cfg
    from contextlib import ExitStack

    import concourse.bass as bass
    import concourse.bacc as bacc
    import concourse.mybir as mybir
    import concourse.tile as tile
    from concourse.masks import make_identity

    dt = mybir.dt
    AF = mybir.ActivationFunctionType
    OP = mybir.AluOpType
    AX = mybir.AxisListType
    f32 = dt.float32
    bf16 = dt.bfloat16

    nc = bacc.Bacc("TRN2", target_bir_lowering=False, debug=False,
                   enable_asserts=False, num_devices=N_CORES)

    # ---- DRAM I/O ----
    lhs32_d = nc.dram_tensor("lhs32", [BL, L, D], f32, kind="ExternalInput").ap()
    vmT_d = nc.dram_tensor("vmT", [BL, L, M], f32, kind="ExternalInput").ap()
    attnw_d = nc.dram_tensor("attnw", [D], f32, kind="ExternalInput").ap()
    qkvw_d = nc.dram_tensor("qkvw", [NL, KD, P, 3 * D], bf16, kind="ExternalInput").ap()
    outw_d = nc.dram_tensor("outw", [NL, KD, P, D], bf16, kind="ExternalInput").ap()
    ff1w_d = nc.dram_tensor("ff1w", [NL, KD, P, DFF], bf16, kind="ExternalInput").ap()
    ff2w_d = nc.dram_tensor("ff2w", [NL, KF, P, D], bf16, kind="ExternalInput").ap()
    w1T_d = nc.dram_tensor("w1T", [KD, P, 100], bf16, kind="ExternalInput").ap()
    w2a_d = nc.dram_tensor("w2a", [NCH, P, CCH], bf16, kind="ExternalInput").ap()
    qkvb_d = outb_d = ff2b_d = ff1b_d = None
    ln1w_d = ln1b_d = ln2w_d = ln2b_d = None
    if qkb_nz or vb_nz:
        qkvb_d = nc.dram_tensor("qkvb", [NL, 3 * D], f32, kind="ExternalInput").ap()
    if f1b_nz:
        ff1b_d = nc.dram_tensor("ff1b", [NL, DFF], f32, kind="ExternalInput").ap()
    if outb_nz:
        outb_d = nc.dram_tensor("outb", [NL, D], f32, kind="ExternalInput").ap()
    if ff2b_nz:
        ff2b_d = nc.dram_tensor("ff2b", [NL, D], f32, kind="ExternalInput").ap()
    if ln1_nt:
        ln1w_d = nc.dram_tensor("ln1w", [NL, D], f32, kind="ExternalInput").ap()
        ln1b_d = nc.dram_tensor("ln1b", [NL, D], f32, kind="ExternalInput").ap()
    if ln2_nt:
        ln2w_d = nc.dram_tensor("ln2w", [NL, D], f32, kind="ExternalInput").ap()
        ln2b_d = nc.dram_tensor("ln2b", [NL, D], f32, kind="ExternalInput").ap()
    out_d = nc.dram_tensor("out2", [NCH, MT, P, CCH], bf16,
                           kind="ExternalOutput").ap()
    xdbg_d = None
    if debug:
        xdbg_d = nc.dram_tensor("xdbg", [NL + 1, BL, M, D], f32,
                                kind="ExternalOutput").ap()

    def bcast_ap(ap, parts):
        return bass.AP(tensor=ap.tensor, offset=ap.offset,
                       ap=[[0, parts]] + [list(x) for x in ap.ap])

    def free_bcast(ap, n):
        """append a stride-0 free dim of size n"""
        return bass.AP(tensor=ap.tensor, offset=ap.offset,
                       ap=[list(x) for x in ap.ap] + [[0, n]])

    def evict_copy(idx, out_ap, in_ap):
        if idx % 2 == 0:
            nc.scalar.copy(out_ap, in_ap)
        else:
            nc.vector.tensor_copy(out_ap, in_ap)

    def mm_splits(nc_, ps, lhsT, rhs, splits, start, stop):
        """n-split matmuls sharing one stationary operand: skip the
        redundant LDWEIGHTS (and its drain bubble) on the later splits"""
        for j, (n0, n1) in enumerate(splits):
            mi = nc_.tensor.matmul(ps[:, n0:n1], lhsT, rhs[:, n0:n1],
                                   start=start, stop=stop)
            if j > 0:
                mi.ins.ldweights = False

    with tile.TileContext(nc) as tc, ExitStack() as ctx:
        const = ctx.enter_context(tc.tile_pool(name="const", bufs=1))
        pools = ctx.enter_context(tc.tile_pool(name="pools", bufs=2))
        xpool = ctx.enter_context(tc.tile_pool(name="xpool", bufs=7))
        xTp = ctx.enter_context(tc.tile_pool(name="xTp", bufs=8))
        qkTp = ctx.enter_context(tc.tile_pool(name="qkTp", bufs=13))
        aoTp = ctx.enter_context(tc.tile_pool(name="aoTp", bufs=7))
        hTp = ctx.enter_context(tc.tile_pool(name="hTp", bufs=25))
        vp = ctx.enter_context(tc.tile_pool(name="vp", bufs=3))
        ap4 = ctx.enter_context(tc.tile_pool(name="ap4", bufs=6))
        stat = ctx.enter_context(tc.tile_pool(name="stat", bufs=12))
        wq = ctx.enter_context(tc.tile_pool(name="wq", bufs=6))
        wo = ctx.enter_context(tc.tile_pool(name="wo", bufs=7))
        wf1 = ctx.enter_context(tc.tile_pool(name="wf1", bufs=6))
        wf2 = ctx.enter_context(tc.tile_pool(name="wf2", bufs=13))
        psS = ctx.enter_context(tc.tile_pool(name="psS", bufs=2, space="PSUM"))
        psT = ctx.enter_context(tc.tile_pool(name="psT", bufs=2, space="PSUM"))
        psW = ctx.enter_context(tc.tile_pool(name="psW", bufs=2, space="PSUM"))
        # pooling-phase-only tiles live in their own pool, released before
        # the classifier pools are allocated so the space is reused
        setup = tc.alloc_tile_pool(name="setup", bufs=1)

        # ---- input + layer-0 weight DMAs first (gpsimd + sync queues) ----
        lhs32_sb = setup.tile([L, BL, D], f32, tag="lhs32", name="lhs32")
        vmT_sb = setup.tile([L, BL, M], f32, tag="vmT", name="vmT")
        for b in range(BL):
            nc.gpsimd.dma_start(out=lhs32_sb[:, b, :], in_=lhs32_d[b])
            nc.gpsimd.dma_start(out=vmT_sb[:, b, :], in_=vmT_d[b])
        attnw_sb = setup.tile([L, D], f32, tag="attnw", name="attnw")
        nc.gpsimd.dma_start(out=attnw_sb[:], in_=bcast_ap(attnw_d, L))

        def load_layer_weights(i):
            qkvw_t = [wq.tile([P, 3 * D], bf16, tag="qkvw", name="qkvw")
                      for _ in range(KD)]
            if i == 0:
                # split so the q,k halves land first and layer-0 qkT starts early
                for ko in range(KD):
                    nc.sync.dma_start(out=qkvw_t[ko][:, 0:2 * D],
                                      in_=qkvw_d[i, ko][:, 0:2 * D])
                for ko in range(KD):
                    nc.sync.dma_start(out=qkvw_t[ko][:, 2 * D:3 * D],
                                      in_=qkvw_d[i, ko][:, 2 * D:3 * D])
            else:
                for ko in range(KD):
                    nc.sync.dma_start(out=qkvw_t[ko][:], in_=qkvw_d[i, ko])
            outw_t = [wo.tile([P, D], bf16, tag="outw", name="outw")
                      for _ in range(KD)]
            for ko in range(KD):
                nc.sync.dma_start(out=outw_t[ko][:], in_=outw_d[i, ko])
            ff1w_t = [wf1.tile([P, DFF], bf16, tag="ff1w", name="ff1w")
                      for _ in range(KD)]
            for ko in range(KD):
                nc.sync.dma_start(out=ff1w_t[ko][:], in_=ff1w_d[i, ko])
            # ff2w streams through its ring during the layer; keep its DMAs
            # off the sync queue (they'd serialize behind the other weights)
            ff2w_t = [wf2.tile([P, D], bf16, tag="ff2w", name="ff2w")
                      for _ in range(KF)]
            for ko in range(KF):
                nc.gpsimd.dma_start(out=ff2w_t[ko][:], in_=ff2w_d[i, ko])
            return qkvw_t, outw_t, ff1w_t, ff2w_t

        layer0_w = load_layer_weights(0)

        # ---- constants ----
        idf = const.tile([P, P], f32, tag="idf", name="idf")
        make_identity(nc, idf[:])
        idb = const.tile([P, P], bf16, tag="idb", name="idb")
        make_identity(nc, idb[:])
        ones32 = const.tile([L, 1], f32, tag="ones32", name="ones32")
        nc.vector.memset(ones32[:], 1.0)
        epst = const.tile([P, 1], f32, tag="epst", name="epst")
        nc.vector.memset(epst[:], EPS)
        qkvb_sb = None
        if qkb_nz:
            qkvb_sb = const.tile([P, NL, 2 * KD], f32, tag="qkvb", name="qkvb")
            for i in range(NL):
                nc.gpsimd.dma_start(
                    out=qkvb_sb[:, i, :],
                    in_=qkvb_d[i, 0:2 * D].rearrange("(t p) -> p t", p=P))
        ff1b_sb = None
        if f1b_nz:
            ff1b_sb = const.tile([P, NL, KF], f32, tag="ff1b", name="ff1b")
            for i in range(NL):
                nc.gpsimd.dma_start(
                    out=ff1b_sb[:, i, :],
                    in_=ff1b_d[i].rearrange("(t p) -> p t", p=P))
        w1T_sb = const.tile([P, KD, 100], bf16, tag="w1T", name="w1T")
        for ko in range(KD):
            nc.sync.dma_start(out=w1T_sb[:, ko, :], in_=w1T_d[ko])

        # ---- mention pooling ----
        x_t = [xpool.tile([P, D], f32, tag="x", name="x") for _ in range(MT)]
        for b in range(BL):
            tmp = setup.tile([L, D], f32, tag="ptmp", name="ptmp", bufs=2)
            nc.vector.tensor_mul(tmp[:], lhs32_sb[:, b, :], attnw_sb[:])
            u = stat.tile([L, 1], f32, tag="u", name="u")
            nc.vector.tensor_reduce(u[:], tmp[:], axis=AX.X, op=OP.add)
            expT = setup.tile([L, M], f32, tag="pexp", name="pexp", bufs=2)
            nc.scalar.activation(expT[:], vmT_sb[:, b, :], AF.Exp,
                                 bias=float(attn_b_val), scale=u[:])
            wun = setup.tile([L, M], f32, tag="pwun", name="pwun", bufs=2)
            nc.vector.tensor_mul(wun[:], expT[:], vmT_sb[:, b, :])
            ps_d = psS.tile([P, 1], f32, tag="s", name="s")
            nc.tensor.matmul(ps_d[:], expT[:], ones32[:], start=True, stop=True)
            r = stat.tile([P, 1], f32, tag="r", name="r")
            nc.vector.reciprocal(r[:], ps_d[:])
            ps_x = psW.tile([P, D], f32, tag="w", name="w")
            for n0, n1 in ((0, 512), (512, D)):
                nc.tensor.matmul(ps_x[:, n0:n1], wun[:], lhs32_sb[:, b, n0:n1],
                                 start=True, stop=True)
            nc.vector.tensor_scalar_mul(x_t[b][:], ps_x[:], r[:])

        if debug:
            for b in range(BL):
                nc.sync.dma_start(out=xdbg_d[0, b], in_=x_t[b][:])

        # pooling inputs no longer needed; reuse their SBUF space for the
        # classifier pools (w2 prefetch ring + output staging)
        setup.release()
        w2p = ctx.enter_context(tc.tile_pool(name="w2p", bufs=W2_PREFETCH + 2))
        ostp = ctx.enter_context(tc.tile_pool(name="ostp", bufs=4))

        # ---- helpers ----
        def transpose_cast(xt):
            """token-major f32 [128,768] x MT -> feature-major bf16 6x[128, 256]"""
            outs = [xTp.tile([P, P * MT], bf16, tag="xT", name="xT") for _ in range(KD)]
            for mo in range(MT):
                for ko in range(KD):
                    ps = psT.tile([P, P], f32, tag="t", name="t")
                    nc.tensor.transpose(ps[:], xt[mo][:, ko * P:(ko + 1) * P], idf[:])
                    nc.vector.tensor_copy(outs[ko][:, mo * P:(mo + 1) * P], ps[:])
            return outs

        def layernorm1(xin_mo, w_bc, b_bc):
            st = stat.tile([P, 2, 6], f32, tag="bns", name="bns")
            for s in range(2):
                nc.vector.bn_stats(st[:, s, :], xin_mo[:, s * 384:(s + 1) * 384])
            mv = stat.tile([P, 2], f32, tag="mv", name="mv")
            nc.vector.bn_aggr(mv[:], st[:])
            std = stat.tile([P, 1], f32, tag="sd", name="sd")
            nc.scalar.activation(std[:], mv[:, 1:2], AF.Sqrt,
                                 bias=epst[:], scale=1.0)
            rstd = stat.tile([P, 1], f32, tag="rs", name="rs")
            nc.vector.reciprocal(rstd[:], std[:])
            nms = stat.tile([P, 1], f32, tag="ns", name="ns")
            nc.vector.tensor_scalar_mul(nms[:], mv[:, 0:1], -1.0)
            nc.vector.tensor_mul(nms[:], nms[:], rstd[:])
            xo = xpool.tile([P, D], f32, tag="x", name="x")
            nc.vector.tensor_scalar(xo[:], xin_mo[:], rstd[:], nms[:],
                                    op0=OP.mult, op1=OP.add)
            if w_bc is not None:
                nc.vector.tensor_mul(xo[:], xo[:], w_bc[:])
            if b_bc is not None:
                nc.vector.tensor_add(xo[:], xo[:], b_bc[:])
            return xo

        w2_tiles = {}

        def load_w2(ci):
            if ci < NCH and ci not in w2_tiles:
                w2t = w2p.tile([P, CCH], bf16, tag="w2", name="w2t")
                nc.sync.dma_start(out=w2t[:], in_=w2a_d[ci])
                w2_tiles[ci] = w2t

        # ---- transformer layers ----
        for i in range(NL):
            qkvw_t, outw_t, ff1w_t, ff2w_t = (layer0_w if i == 0
                                              else load_layer_weights(i))
            vb_bc = None
            if vb_nz:
                vb_bc = pools.tile([P, D], f32, tag="vbb", name="vbb")
                nc.gpsimd.dma_start(out=vb_bc[:],
                                    in_=bcast_ap(qkvb_d[i, 2 * D:3 * D], P))
            outb_bc = None
            if outb_nz:
                outb_bc = pools.tile([P, D], f32, tag="obb", name="obb")
                nc.gpsimd.dma_start(out=outb_bc[:], in_=bcast_ap(outb_d[i], P))
            ff2b_bc = None
            if ff2b_nz:
                ff2b_bc = pools.tile([P, D], f32, tag="fbb", name="fbb")
                nc.gpsimd.dma_start(out=ff2b_bc[:], in_=bcast_ap(ff2b_d[i], P))
            ln1w_bc = ln1b_bc = ln2w_bc = ln2b_bc = None
            if ln1_nt:
                ln1w_bc = pools.tile([P, D], f32, tag="l1w", name="l1w")
                nc.gpsimd.dma_start(out=ln1w_bc[:], in_=bcast_ap(ln1w_d[i], P))
                ln1b_bc = pools.tile([P, D], f32, tag="l1b", name="l1b")
                nc.gpsimd.dma_start(out=ln1b_bc[:], in_=bcast_ap(ln1b_d[i], P))
            if ln2_nt:
                ln2w_bc = pools.tile([P, D], f32, tag="l2w", name="l2w")
                nc.gpsimd.dma_start(out=ln2w_bc[:], in_=bcast_ap(ln2w_d[i], P))
                ln2b_bc = pools.tile([P, D], f32, tag="l2b", name="l2b")
                nc.gpsimd.dma_start(out=ln2b_bc[:], in_=bcast_ap(ln2b_d[i], P))

            # preload the Exp activation table while the PE does transposes,
            # so the first attention exp doesn't stall on ACT_TABLE_LOAD
            dum = stat.tile([P, 1], f32, tag="dum", name="dum")
            nc.scalar.activation(dum[:], epst[:], AF.Exp, scale=1.0)

            xT = transpose_cast(x_t)

            # q,k feature-major [1536, 256]; emit q/k tile pairs interleaved
            # so attention head 0 (needs tiles 0 and 6) can start early
            qkT = [None] * NQK
            for gi, mo12 in enumerate(t for p in zip(range(KD), range(KD, NQK))
                                      for t in p):
                ps = psS.tile([P, P * MT], f32, tag="s", name="s")
                for ko in range(KD):
                    nc.tensor.matmul(ps[:], qkvw_t[ko][:, mo12 * P:(mo12 + 1) * P],
                                     xT[ko][:], start=(ko == 0), stop=(ko == KD - 1))
                t = qkTp.tile([P, P * MT], bf16, tag="qkT", name="qkT")
                if qkb_nz:
                    if gi % 2 == 0:
                        nc.scalar.activation(t[:], ps[:], AF.Identity,
                                             bias=qkvb_sb[:, i, mo12:mo12 + 1],
                                             scale=1.0)
                    else:
                        nc.vector.tensor_scalar_add(t[:], ps[:],
                                                    qkvb_sb[:, i, mo12:mo12 + 1])
                else:
                    nc.vector.tensor_copy(t[:], ps[:])
                qkT[mo12] = t

            # v token-major [256, 768] with interleaved ones columns per head:
            # v_aug [128, 12, 65] (col 64 of each head = 1.0)
            v_aug = [vp.tile([P, H, HD + 1], bf16, tag="vaug", name="vaug")
                     for _ in range(MT)]
            ps_v = [psW.tile([P, D], f32, tag="w", name="w") for _ in range(MT)]
            for ko in range(KD):
                for mo in range(MT):
                    mm_splits(nc, ps_v[mo], xT[ko][:, mo * P:(mo + 1) * P],
                              qkvw_t[ko][:, 2 * D:3 * D],
                              ((0, 512), (512, D)),
                              ko == 0, ko == KD - 1)
            for mo in range(MT):
                nc.vector.memset(v_aug[mo][:, :, HD:HD + 1], 1.0)
                src = ps_v[mo][:].rearrange("p (h d) -> p h d", d=HD)
                if vb_nz:
                    vb3 = vb_bc[:].rearrange("p (h d) -> p h d", d=HD)
                    nc.vector.scalar_tensor_tensor(
                        v_aug[mo][:, :, 0:HD], src, 1.0, vb3,
                        op0=OP.mult, op1=OP.add)
                else:
                    nc.vector.tensor_copy(v_aug[mo][:, :, 0:HD], src)

            # attention per (batch, head): scores computed k-major
            # (lhsT=k, rhs=q) so exp feeds AV directly as lhsT; AV output
            # [q, 65] per head packs into 2 psum tiles per batch; softmax
            # normalization batched into 2 recip + 2 strided muls.
            ao_bf = [vp.tile([P, D], bf16, tag="ao", name="ao") for _ in range(MT)]
            for b in range(MT):
                ps_ao_a = psW.tile([P, D], f32, tag="w", name="w")
                ps_ao_b = psW.tile([P, D], f32, tag="w", name="w")
                pk_a = ps_ao_a[:, 0:7 * (HD + 1)].rearrange(
                    "p (h c) -> p h c", c=HD + 1)
                pk_b = ps_ao_b[:, 0:5 * (HD + 1)].rearrange(
                    "p (h c) -> p h c", c=HD + 1)
                # software-pipelined: AV(h) issues 2 heads behind scores(h)
                # so the PE never waits the scores->exp scalar latency
                LAG = 3
                exs = [None] * H
                for idx in range(H + LAG):
                    if idx < H:
                        h = idx
                        t_idx, row0 = h // 2, (h % 2) * HD
                        q_ap = qkT[t_idx][row0:row0 + HD, b * P:(b + 1) * P]
                        k_ap = qkT[KD + t_idx][row0:row0 + HD, b * P:(b + 1) * P]
                        pool_s = psT if h % 2 == 0 else psS
                        ps_s = pool_s.tile([P, P], f32, tag=("t" if h % 2 == 0
                                                             else "s"), name="s")
                        nc.tensor.matmul(ps_s[:], k_ap, q_ap, start=True, stop=True)
                        ex = ap4.tile([P, P], bf16, tag="abf", name="abf")
                        nc.scalar.activation(ex[:], ps_s[:], AF.Exp,
                                             scale=1.0 / np.sqrt(HD))
                        exs[h] = ex
                    if idx >= LAG:
                        h = idx - LAG
                        pk = pk_a[:, h, :] if h < 7 else pk_b[:, h - 7, :]
                        va = v_aug[b][:, h, :]
                        nc.tensor.matmul(pk, exs[h][:], va, start=True, stop=True)
                # batched softmax normalization
                rec = stat.tile([P, H], f32, tag="rec", name="rec")
                nc.vector.reciprocal(rec[:, 0:7], pk_a[:, :, HD])
                nc.vector.reciprocal(rec[:, 7:H], pk_b[:, :, HD])
                ao3a = ao_bf[b][:, 0:7 * HD].rearrange("p (h d) -> p h d", d=HD)
                ao3b = ao_bf[b][:, 7 * HD:D].rearrange("p (h d) -> p h d", d=HD)
                nc.vector.tensor_mul(ao3a, pk_a[:, :, 0:HD],
                                     free_bcast(rec[:, 0:7], HD))
                nc.vector.tensor_mul(ao3b, pk_b[:, :, 0:HD],
                                     free_bcast(rec[:, 7:H], HD))

            # transpose ao to feature-major for the out-projection
            aoT = [aoTp.tile([P, P * MT], bf16, tag="aoT", name="aoT") for _ in range(KD)]
            for mo in range(MT):
                for ko in range(KD):
                    ps = psT.tile([P, P], bf16, tag="t", name="t")
                    nc.tensor.transpose(ps[:], ao_bf[mo][:, ko * P:(ko + 1) * P],
                                        idb[:])
                    nc.vector.tensor_copy(aoT[ko][:, mo * P:(mo + 1) * P], ps[:])

            # out-proj + residual + LN1 + re-transpose, mo-outer so batch 0's
            # vector-side LN overlaps batch 1's matmuls
            x1n_t = [None] * MT
            x1nT = [xTp.tile([P, P * MT], bf16, tag="xT", name="xT")
                    for _ in range(KD)]
            for mo in range(MT):
                ps_o = psW.tile([P, D], f32, tag="w", name="w")
                for ko in range(KD):
                    mm_splits(nc, ps_o, aoT[ko][:, mo * P:(mo + 1) * P],
                              outw_t[ko][:], ((0, 512), (512, D)),
                              ko == 0, ko == KD - 1)
                x1 = xpool.tile([P, D], f32, tag="x", name="x")
                nc.vector.scalar_tensor_tensor(
                    x1[:], ps_o[:], 1.0, x_t[mo][:],
                    op0=OP.mult, op1=OP.add)
                if outb_nz:
                    nc.vector.tensor_add(x1[:], x1[:], outb_bc[:])
                x1n_t[mo] = layernorm1(x1, ln1w_bc, ln1b_bc)
            for mo in range(MT):
                for ko in range(KD):
                    ps = psT.tile([P, P], f32, tag="t", name="t")
                    nc.tensor.transpose(ps[:], x1n_t[mo][:, ko * P:(ko + 1) * P],
                                        idf[:])
                    nc.vector.tensor_copy(x1nT[ko][:, mo * P:(mo + 1) * P], ps[:])

            # ff1 (relu) feature-major [3072, 256]
            hT = [hTp.tile([P, P * MT], bf16, tag="hT", name="hT") for _ in range(KF)]
            for mo24 in range(KF):
                ps = psS.tile([P, P * MT], f32, tag="s", name="s")
                for ko in range(KD):
                    nc.tensor.matmul(ps[:], ff1w_t[ko][:, mo24 * P:(mo24 + 1) * P],
                                     x1nT[ko][:], start=(ko == 0), stop=(ko == KD - 1))
                if f1b_nz:
                    if mo24 % 2 == 0:
                        nc.scalar.activation(hT[mo24][:], ps[:], AF.Relu,
                                             bias=ff1b_sb[:, i, mo24:mo24 + 1],
                                             scale=1.0)
                    else:
                        nc.vector.tensor_scalar(hT[mo24][:], ps[:],
                                                ff1b_sb[:, i, mo24:mo24 + 1], 0.0,
                                                op0=OP.add, op1=OP.max)
                else:
                    if mo24 % 2 == 0:
                        nc.scalar.activation(hT[mo24][:], ps[:], AF.Relu,
                                             scale=1.0)
                    else:
                        nc.vector.tensor_scalar_max(hT[mo24][:], ps[:], 0.0)

            # ff2 + residual + LN2. ko-outer so each ff2w tile's last read is
            # immediate and the 8-slot ring streams (mo-outer would pin all 24
            # tiles live and serialize the weight DMAs against the matmuls).
            x_t = [None] * MT
            ps_y = [psW.tile([P, D], f32, tag="w", name="w") for _ in range(MT)]
            KH = KF // 2
            for ko in range(KH):
                for mo in range(MT):
                    mm_splits(nc, ps_y[mo], hT[ko][:, mo * P:(mo + 1) * P],
                              ff2w_t[ko][:], ((0, 512), (512, D)),
                              ko == 0, False)
            # finish mo=0 first so its residual+LN overlaps mo=1's matmuls
            for mo in range(MT):
                for ko in range(KH, KF):
                    mm_splits(nc, ps_y[mo], hT[ko][:, mo * P:(mo + 1) * P],
                              ff2w_t[ko][:], ((0, 512), (512, D)),
                              False, ko == KF - 1)
                x2 = xpool.tile([P, D], f32, tag="x", name="x")
                nc.vector.scalar_tensor_tensor(
                    x2[:], ps_y[mo][:], 1.0, x1n_t[mo][:],
                    op0=OP.mult, op1=OP.add)
                if ff2b_nz:
                    nc.vector.tensor_add(x2[:], x2[:], ff2b_bc[:])
                x_t[mo] = layernorm1(x2, ln2w_bc, ln2b_bc)
            if debug:
                for b in range(BL):
                    nc.sync.dma_start(out=xdbg_d[i + 1, b], in_=x_t[b][:])

            # spread classifier w2 prefetch over layers 1..3 to smooth the
            # HBM demand (classifier phase alone would oversubscribe DMA)
            if i >= 1:
                n0 = (i - 1) * 6
                n1 = i * 6 if i < NL - 1 else W2_PREFETCH
                for ci in range(n0, n1):
                    load_w2(ci)

        # ---- classifier ----
        xT = transpose_cast(x_t)
        ps_h = psS.tile([P, P * MT], f32, tag="s", name="s")
        for ko in range(KD):
            nc.tensor.matmul(ps_h[0:100, :], w1T_sb[:, ko, :], xT[ko][:],
                             start=(ko == 0), stop=(ko == KD - 1))
        hTa = const.tile([P, P * MT], bf16, tag="hTa", name="hTa")
        nc.vector.memset(hTa[:, :], 1.0)
        nc.vector.tensor_copy(hTa[0:100, :], ps_h[0:100, :])

        # logits: f32 psum halves (one bank each), bf16 evictions into a
        # 1024-wide bf16 tile, chunk-major output. psum slots alternate
        # between psS and psW; evictions split Scalar/Vector; out DMAs
        # alternate Sync/Scalar queues.
        HC = CCH // 2
        cseq = ((psS, "s"), (psT, "t"), (psW, "w"))
        cj = 0
        for ci in range(NCH):
            load_w2(ci + W2_PREFETCH)
            w2t = w2_tiles.pop(ci)
            for mo in range(MT):
                p0, tg0 = cseq[cj % 3]
                p1, tg1 = cseq[(cj + 1) % 3]
                cj += 2
                ps0 = p0.tile([P, HC], f32, tag=tg0, name="s")
                ps1 = p1.tile([P, HC], f32, tag=tg1, name="s")
                lhs = hTa[:, mo * P:(mo + 1) * P]
                nc.tensor.matmul(ps0[:], lhs, w2t[:, 0:HC], start=True, stop=True)
                mi = nc.tensor.matmul(ps1[:], lhs, w2t[:, HC:CCH],
                                      start=True, stop=True)
                mi.ins.ldweights = False
                ost = ostp.tile([P, CCH], bf16, tag="ost", name="ost")
                nc.scalar.copy(ost[:, 0:HC], ps0[:])
                nc.vector.tensor_copy(ost[:, HC:CCH], ps1[:])
                eng = nc.sync if mo == 0 else nc.gpsimd
                eng.dma_start(out=out_d[ci, mo], in_=ost[:])

    nc.compile()
    return nc


def _chunk_w2(cls_w2, cls_b2):
    # rows: 100 weights + 1 bias + 27 zero pad (lhsT rows 101.. are 1.0 from
    # the hTa memset, so the zero rows contribute nothing)
    w2a = np.concatenate(
        [cls_w2.T, cls_b2[None, :], np.zeros((27, NE), np.float32)], axis=0
    ).astype(BF16)  # [128, NE]
    pad = NCH * CCH - NE
    if pad:
        w2a = np.concatenate([w2a, np.zeros((128, pad), BF16)], axis=1)
    return np.ascontiguousarray(w2a.reshape(128, NCH, CCH).transpose(1, 0, 2))


def _prep(inputs):
    lhs = np.asarray(inputs["last_hidden_state"], dtype=np.float32)
    pos = np.asarray(inputs["entity_position_ids"])
    msk = np.asarray(inputs["entity_attention_mask"])
    qkv_w = np.asarray(inputs["qkv_w"], dtype=np.float32)
    qkv_b = np.asarray(inputs["qkv_b"], dtype=np.float32)
    out_w = np.asarray(inputs["out_w"], dtype=np.float32)
    out_b = np.asarray(inputs["out_b"], dtype=np.float32)
    ln1_w = np.asarray(inputs["ln1_w"], dtype=np.float32)
    ln1_b = np.asarray(inputs["ln1_b"], dtype=np.float32)
    ff1_w = np.asarray(inputs["ff1_w"], dtype=np.float32)
    ff1_b = np.asarray(inputs["ff1_b"], dtype=np.float32)
    ff2_w = np.asarray(inputs["ff2_w"], dtype=np.float32)
    ff2_b = np.asarray(inputs["ff2_b"], dtype=np.float32)
    ln2_w = np.asarray(inputs["ln2_w"], dtype=np.float32)
    ln2_b = np.asarray(inputs["ln2_b"], dtype=np.float32)
    cls_w1 = np.asarray(inputs["cls_w1"], dtype=np.float32)
    cls_w2 = np.asarray(inputs["cls_w2"], dtype=np.float32)
    cls_b2 = np.asarray(inputs["cls_b2"], dtype=np.float32)
    attn_w = np.asarray(inputs["attn_w"], dtype=np.float32)
    attn_b = float(np.asarray(inputs["attn_b"], dtype=np.float32))

    # ragged valid mask: 1 up to the first -1 (and only where attention mask set)
    nb = np.cumprod((pos != -1).astype(np.int32), axis=-1)
    valid = (msk != 0).astype(np.int32)[:, :, None] * nb       # [B, M, L]
    vmT = np.ascontiguousarray(valid.transpose(0, 2, 1)).astype(np.float32)

    cfg = (
        attn_b,
        bool(np.any(qkv_b[:, :2 * D])),
        bool(np.any(qkv_b[:, 2 * D:])),
        bool(np.any(out_b)),
        bool(np.any(ff1_b)),
        bool(np.any(ff2_b)),
        not (np.all(ln1_w == 1.0) and np.all(ln1_b == 0.0)),
        not (np.all(ln2_w == 1.0) and np.all(ln2_b == 0.0)),
        bool(KERNEL_DEBUG),
    )

    shared = {
        "attnw": attn_w,
        "qkvw": np.ascontiguousarray(qkv_w.transpose(0, 2, 1)).reshape(
            NL, KD, P, 3 * D).astype(BF16),
        "outw": np.ascontiguousarray(out_w.transpose(0, 2, 1)).reshape(
            NL, KD, P, D).astype(BF16),
        "ff1w": np.ascontiguousarray(ff1_w.transpose(0, 2, 1)).reshape(
            NL, KD, P, DFF).astype(BF16),
        "ff2w": np.ascontiguousarray(ff2_w.transpose(0, 2, 1)).reshape(
            NL, KF, P, D).astype(BF16),
        "w1T": np.ascontiguousarray(cls_w1.T).reshape(KD, P, 100).astype(BF16),
        "w2a": _chunk_w2(cls_w2, cls_b2),
    }
    if cfg[1] or cfg[2]:
        shared["qkvb"] = qkv_b
    if cfg[3]:
        shared["outb"] = out_b
    if cfg[4]:
        shared["ff1b"] = ff1_b
    if cfg[5]:
        shared["ff2b"] = ff2_b
    if cfg[6]:
        shared["ln1w"] = ln1_w
        shared["ln1b"] = ln1_b
    if cfg[7]:
        shared["ln2w"] = ln2_w
        shared["ln2b"] = ln2_b

    lhs32 = np.ascontiguousarray(lhs[:, :L, :])
    in_maps = []
    for c in range(N_CORES):
        m = dict(shared)
        m["lhs32"] = np.ascontiguousarray(lhs32[c * BL:(c + 1) * BL])
        m["vmT"] = np.ascontiguousarray(vmT[c * BL:(c + 1) * BL])
        in_maps.append(m)
    return cfg, in_maps


def kernel(**inputs):
    from concourse.bass_utils import run_bass_kernel_spmd

    cfg, in_maps = _prep(inputs)
    if cfg not in _CACHE:
        _CACHE[cfg] = _build(cfg)
    nc = _CACHE[cfg]
    res = run_bass_kernel_spmd(nc, in_maps, core_ids=list(range(N_CORES)))
    parts = []
    for c in range(N_CORES):
        o = res.results[c]["out2"]  # [NCH, MT, P, CCH] bf16
        o = np.ascontiguousarray(o.transpose(1, 2, 0, 3)).reshape(BL, M, NCH * CCH)
        parts.append(o[:, :, :NE].astype(np.float32))
    out = np.concatenate(parts, axis=0)
    if KERNEL_DEBUG:
        kernel.last_debug = [res.results[c].get("xdbg") for c in range(N_CORES)]
    return out


# revision 35
# speedup vs baseline: 1.0036x; 1.0036x over previous
"""Trainium2 Bass kernel for nn_EnokeeEncoder (ragged mention pooling +
4-layer transformer + 50k-entity classifier), data-parallel over batch
across 8 NeuronCores.

Layout strategy per core (2 batches, 256 mention-tokens):
  - residual stream x: token-major [128 tokens/p, 768] f32 (LN/softmax easy)
  - matmul chains run feature-major via PE transposes of x
  - all big matmuls in bf16 (weights pre-cast+pre-transposed on host),
    f32 accumulation in PSUM; LN / softmax / residual in f32.
  - attention scores computed k-major (lhsT=k, rhs=q) so exp output feeds
    the AV matmul directly as the stationary operand - no transpose.
  - AV matmuls append a ones-column per head (v_aug) giving the softmax
    denominator; all 12 heads of a batch pack into two PSUM tiles so the
    normalization is 2 reciprocals + 2 strided tensor_tensor muls.
  - classifier: f32 psum halves (512-wide, one bank each) rotated across
    all three psum pools, evicted to a 1024-wide bf16 tile, chunk-major
    DRAM layout, unpacked/cast to f32 on host; w2 chunks prefetched into
    SBUF during layers 1-3 to smooth HBM demand.
"""

import sys

for _p in ("/opt/trn_rl_repo",):
    if _p not in sys.path:
        sys.path.insert(0, _p)

import numpy as np
import ml_dtypes

BF16 = ml_dtypes.bfloat16

B, M, L, S = 16, 128, 32, 512
D, H, DFF, NL = 768, 12, 3072, 4
NE = 50000
HD = D // H
EPS = 1e-5
N_CORES = 8
BL = B // N_CORES          # batches per core
P = 128
KD = D // P                # 6 k-tiles over D
KF = DFF // P              # 24 k-tiles over DFF
MT = BL                    # token m-tiles per core (M == P)
NQK = 2 * D // P           # 12 m-tiles over q,k features
CCH = 1024                 # classifier chunk (bf16 psum bank)
NCH = (NE + CCH - 1) // CCH  # 49
W2_PREFETCH = 17

KERNEL_DEBUG = False
_CACHE = {}


def _build(cfg):
    (attn_b_val, qkb_nz, vb_nz, outb_nz, f1b_nz, ff2b_nz, ln1_nt, ln2_nt,
     debug) = cfg
    from contextlib import ExitStack

    import concourse.bass as bass
    import concourse.bacc as bacc
    import concourse.mybir as mybir
    import concourse.tile as tile
    from concourse.masks import make_identity

    dt = mybir.dt
    AF = mybir.ActivationFunctionType
    OP = mybir.AluOpType
    AX = mybir.AxisListType
    f32 = dt.float32
    bf16 = dt.bfloat16

    nc = bacc.Bacc("TRN2", target_bir_lowering=False, debug=False,
                   enable_asserts=False, num_devices=N_CORES)

    # ---- DRAM I/O ----
    lhs32_d = nc.dram_tensor("lhs32", [BL, L, D], f32, kind="ExternalInput").ap()
    vmT_d = nc.dram_tensor("vmT", [BL, L, M], f32, kind="ExternalInput").ap()
    attnw_d = nc.dram_tensor("attnw", [D], f32, kind="ExternalInput").ap()
    qkvw_d = nc.dram_tensor("qkvw", [NL, KD, P, 3 * D], bf16, kind="ExternalInput").ap()
    outw_d = nc.dram_tensor("outw", [NL, KD, P, D], bf16, kind="ExternalInput").ap()
    ff1w_d = nc.dram_tensor("ff1w", [NL, KD, P, DFF], bf16, kind="ExternalInput").ap()
    ff2w_d = nc.dram_tensor("ff2w", [NL, KF, P, D], bf16, kind="ExternalInput").ap()
    w1T_d = nc.dram_tensor("w1T", [KD, P, 100], bf16, kind="ExternalInput").ap()
    w2a_d = nc.dram_tensor("w2a", [NCH, P, CCH], bf16, kind="ExternalInput").ap()
    qkvb_d = outb_d = ff2b_d = ff1b_d = None
    ln1w_d = ln1b_d = ln2w_d = ln2b_d = None
    if qkb_nz or vb_nz:
        qkvb_d = nc.dram_tensor("qkvb", [NL, 3 * D], f32, kind="ExternalInput").ap()
    if f1b_nz:
        ff1b_d = nc.dram_tensor("ff1b", [NL, DFF], f32, kind="ExternalInput").ap()
    if outb_nz:
        outb_d = nc.dram_tensor("outb", [NL, D], f32, kind="ExternalInput").ap()
    if ff2b_nz:
        ff2b_d = nc.dram_tensor("ff2b", [NL, D], f32, kind="ExternalInput").ap()
    if ln1_nt:
        ln1w_d = nc.dram_tensor("ln1w", [NL, D], f32, kind="ExternalInput").ap()
        ln1b_d = nc.dram_tensor("ln1b", [NL, D], f32, kind="ExternalInput").ap()
    if ln2_nt:
        ln2w_d = nc.dram_tensor("ln2w", [NL, D], f32, kind="ExternalInput").ap()
        ln2b_d = nc.dram_tensor("ln2b", [NL, D], f32, kind="ExternalInput").ap()
    out_d = nc.dram_tensor("out2", [NCH, MT, P, CCH], bf16,
                           kind="ExternalOutput").ap()
    xdbg_d = None
    if debug:
        xdbg_d = nc.dram_tensor("xdbg", [NL + 1, BL, M, D], f32,
                                kind="ExternalOutput").ap()

    def bcast_ap(ap, parts):
        return bass.AP(tensor=ap.tensor, offset=ap.offset,
                       ap=[[0, parts]] + [list(x) for x in ap.ap])

    def free_bcast(ap, n):
        """append a stride-0 free dim of size n"""
        return bass.AP(tensor=ap.tensor, offset=ap.offset,
                       ap=[list(x) for x in ap.ap] + [[0, n]])

    def evict_copy(idx, out_ap, in_ap):
        if idx % 2 == 0:
            nc.scalar.copy(out_ap, in_ap)
        else:
            nc.vector.tensor_copy(out_ap, in_ap)

    def mm_splits(nc_, ps, lhsT, rhs, splits, start, stop):
        """n-split matmuls sharing one stationary operand: skip the
        redundant LDWEIGHTS (and its drain bubble) on the later splits"""
        for j, (n0, n1) in enumerate(splits):
            mi = nc_.tensor.matmul(ps[:, n0:n1], lhsT, rhs[:, n0:n1],
                                   start=start, stop=stop)
            if j > 0:
                mi.ins.ldweights = False

    with tile.TileContext(nc) as tc, ExitStack() as ctx:
        const = ctx.enter_context(tc.tile_pool(name="const", bufs=1))
        pools = ctx.enter_context(tc.tile_pool(name="pools", bufs=2))
        xpool = ctx.enter_context(tc.tile_pool(name="xpool", bufs=7))
        xTp = ctx.enter_context(tc.tile_pool(name="xTp", bufs=8))
        qkTp = ctx.enter_context(tc.tile_pool(name="qkTp", bufs=13))
        aoTp = ctx.enter_context(tc.tile_pool(name="aoTp", bufs=7))
        hTp = ctx.enter_context(tc.tile_pool(name="hTp", bufs=25))
        vp = ctx.enter_context(tc.tile_pool(name="vp", bufs=3))
        ap4 = ctx.enter_context(tc.tile_pool(name="ap4", bufs=6))
        stat = ctx.enter_context(tc.tile_pool(name="stat", bufs=12))
        wq = ctx.enter_context(tc.tile_pool(name="wq", bufs=6))
        wo = ctx.enter_context(tc.tile_pool(name="wo", bufs=7))
        wf1 = ctx.enter_context(tc.tile_pool(name="wf1", bufs=6))
        wf2 = ctx.enter_context(tc.tile_pool(name="wf2", bufs=13))
        psS = ctx.enter_context(tc.tile_pool(name="psS", bufs=2, space="PSUM"))
        psT = ctx.enter_context(tc.tile_pool(name="psT", bufs=2, space="PSUM"))
        psW = ctx.enter_context(tc.tile_pool(name="psW", bufs=2, space="PSUM"))
        # pooling-phase-only tiles live in their own pool, released before
        # the classifier pools are allocated so the space is reused
        setup = tc.alloc_tile_pool(name="setup", bufs=1)

        # ---- input + layer-0 weight DMAs first (gpsimd + sync queues) ----
        lhs32_sb = setup.tile([L, BL, D], f32, tag="lhs32", name="lhs32")
        vmT_sb = setup.tile([L, BL, M], f32, tag="vmT", name="vmT")
        for b in range(BL):
            nc.gpsimd.dma_start(out=lhs32_sb[:, b, :], in_=lhs32_d[b])
            nc.gpsimd.dma_start(out=vmT_sb[:, b, :], in_=vmT_d[b])
        attnw_sb = setup.tile([L, D], f32, tag="attnw", name="attnw")
        nc.gpsimd.dma_start(out=attnw_sb[:], in_=bcast_ap(attnw_d, L))

        def load_layer_weights(i):
            qkvw_t = [wq.tile([P, 3 * D], bf16, tag="qkvw", name="qkvw")
                      for _ in range(KD)]
            if i == 0:
                # split so the q,k halves land first and layer-0 qkT starts early
                for ko in range(KD):
                    nc.sync.dma_start(out=qkvw_t[ko][:, 0:2 * D],
                                      in_=qkvw_d[i, ko][:, 0:2 * D])
                for ko in range(KD):
                    nc.sync.dma_start(out=qkvw_t[ko][:, 2 * D:3 * D],
                                      in_=qkvw_d[i, ko][:, 2 * D:3 * D])
            else:
                for ko in range(KD):
                    nc.sync.dma_start(out=qkvw_t[ko][:], in_=qkvw_d[i, ko])
            outw_t = [wo.tile([P, D], bf16, tag="outw", name="outw")
                      for _ in range(KD)]
            for ko in range(KD):
                nc.sync.dma_start(out=outw_t[ko][:], in_=outw_d[i, ko])
            ff1w_t = [wf1.tile([P, DFF], bf16, tag="ff1w", name="ff1w")
                      for _ in range(KD)]
            for ko in range(KD):
                nc.sync.dma_start(out=ff1w_t[ko][:], in_=ff1w_d[i, ko])
            # ff2w streams through its ring during the layer; keep its DMAs
            # off the sync queue (they'd serialize behind the other weights)
            ff2w_t = [wf2.tile([P, D], bf16, tag="ff2w", name="ff2w")
                      for _ in range(KF)]
            for ko in range(KF):
                nc.gpsimd.dma_start(out=ff2w_t[ko][:], in_=ff2w_d[i, ko])
            return qkvw_t, outw_t, ff1w_t, ff2w_t

        layer0_w = load_layer_weights(0)

        # ---- constants ----
        idf = const.tile([P, P], f32, tag="idf", name="idf")
        make_identity(nc, idf[:])
        idb = const.tile([P, P], bf16, tag="idb", name="idb")
        make_identity(nc, idb[:])
        ones32 = const.tile([L, 1], f32, tag="ones32", name="ones32")
        nc.vector.memset(ones32[:], 1.0)
        epst = const.tile([P, 1], f32, tag="epst", name="epst")
        nc.vector.memset(epst[:], EPS)
        qkvb_sb = None
        if qkb_nz:
            qkvb_sb = const.tile([P, NL, 2 * KD], f32, tag="qkvb", name="qkvb")
            for i in range(NL):
                nc.gpsimd.dma_start(
                    out=qkvb_sb[:, i, :],
                    in_=qkvb_d[i, 0:2 * D].rearrange("(t p) -> p t", p=P))
        ff1b_sb = None
        if f1b_nz:
            ff1b_sb = const.tile([P, NL, KF], f32, tag="ff1b", name="ff1b")
            for i in range(NL):
                nc.gpsimd.dma_start(
                    out=ff1b_sb[:, i, :],
                    in_=ff1b_d[i].rearrange("(t p) -> p t", p=P))
        w1T_sb = const.tile([P, KD, 100], bf16, tag="w1T", name="w1T")
        for ko in range(KD):
            nc.sync.dma_start(out=w1T_sb[:, ko, :], in_=w1T_d[ko])

        # ---- mention pooling ----
        x_t = [xpool.tile([P, D], f32, tag="x", name="x") for _ in range(MT)]
        for b in range(BL):
            tmp = setup.tile([L, D], f32, tag="ptmp", name="ptmp", bufs=2)
            nc.vector.tensor_mul(tmp[:], lhs32_sb[:, b, :], attnw_sb[:])
            u = stat.tile([L, 1], f32, tag="u", name="u")
            nc.vector.tensor_reduce(u[:], tmp[:], axis=AX.X, op=OP.add)
            expT = setup.tile([L, M], f32, tag="pexp", name="pexp", bufs=2)
            nc.scalar.activation(expT[:], vmT_sb[:, b, :], AF.Exp,
                                 bias=float(attn_b_val), scale=u[:])
            wun = setup.tile([L, M], f32, tag="pwun", name="pwun", bufs=2)
            nc.vector.tensor_mul(wun[:], expT[:], vmT_sb[:, b, :])
            ps_d = psS.tile([P, 1], f32, tag="s", name="s")
            nc.tensor.matmul(ps_d[:], expT[:], ones32[:], start=True, stop=True)
            r = stat.tile([P, 1], f32, tag="r", name="r")
            nc.vector.reciprocal(r[:], ps_d[:])
            ps_x = psW.tile([P, D], f32, tag="w", name="w")
            for n0, n1 in ((0, 512), (512, D)):
                nc.tensor.matmul(ps_x[:, n0:n1], wun[:], lhs32_sb[:, b, n0:n1],
                                 start=True, stop=True)
            nc.vector.tensor_scalar_mul(x_t[b][:], ps_x[:], r[:])

        if debug:
            for b in range(BL):
                nc.sync.dma_start(out=xdbg_d[0, b], in_=x_t[b][:])

        # pooling inputs no longer needed; reuse their SBUF space for the
        # classifier pools (w2 prefetch ring + output staging)
        setup.release()
        w2p = ctx.enter_context(tc.tile_pool(name="w2p", bufs=W2_PREFETCH + 2))
        ostp = ctx.enter_context(tc.tile_pool(name="ostp", bufs=4))

        # ---- helpers ----
        def transpose_cast(xt):
            """token-major f32 [128,768] x MT -> feature-major bf16 6x[128, 256]"""
            outs = [xTp.tile([P, P * MT], bf16, tag="xT", name="xT") for _ in range(KD)]
            for mo in range(MT):
                for ko in range(KD):
                    ps = psT.tile([P, P], f32, tag="t", name="t")
                    nc.tensor.transpose(ps[:], xt[mo][:, ko * P:(ko + 1) * P], idf[:])
                    nc.vector.tensor_copy(outs[ko][:, mo * P:(mo + 1) * P], ps[:])
            return outs

        def layernorm1(xin_mo, w_bc, b_bc):
            st = stat.tile([P, 2, 6], f32, tag="bns", name="bns")
            for s in range(2):
                nc.vector.bn_stats(st[:, s, :], xin_mo[:, s * 384:(s + 1) * 384])
            mv = stat.tile([P, 2], f32, tag="mv", name="mv")
            nc.vector.bn_aggr(mv[:], st[:])
            std = stat.tile([P, 1], f32, tag="sd", name="sd")
            nc.scalar.activation(std[:], mv[:, 1:2], AF.Sqrt,
                                 bias=epst[:], scale=1.0)
            rstd = stat.tile([P, 1], f32, tag="rs", name="rs")
            nc.vector.reciprocal(rstd[:], std[:])
            nms = stat.tile([P, 1], f32, tag="ns", name="ns")
            nc.vector.tensor_scalar_mul(nms[:], mv[:, 0:1], -1.0)
            nc.vector.tensor_mul(nms[:], nms[:], rstd[:])
            xo = xpool.tile([P, D], f32, tag="x", name="x")
            nc.vector.tensor_scalar(xo[:], xin_mo[:], rstd[:], nms[:],
                                    op0=OP.mult, op1=OP.add)
            if w_bc is not None:
                nc.vector.tensor_mul(xo[:], xo[:], w_bc[:])
            if b_bc is not None:
                nc.vector.tensor_add(xo[:], xo[:], b_bc[:])
            return xo

        w2_tiles = {}

        def load_w2(ci):
            if ci < NCH and ci not in w2_tiles:
                w2t = w2p.tile([P, CCH], bf16, tag="w2", name="w2t")
                nc.sync.dma_start(out=w2t[:], in_=w2a_d[ci])
                w2_tiles[ci] = w2t

        # ---- transformer layers ----
        for i in range(NL):
            qkvw_t, outw_t, ff1w_t, ff2w_t = (layer0_w if i == 0
                                              else load_layer_weights(i))
            vb_bc = None
            if vb_nz:
                vb_bc = pools.tile([P, D], f32, tag="vbb", name="vbb")
                nc.gpsimd.dma_start(out=vb_bc[:],
                                    in_=bcast_ap(qkvb_d[i, 2 * D:3 * D], P))
            outb_bc = None
            if outb_nz:
                outb_bc = pools.tile([P, D], f32, tag="obb", name="obb")
                nc.gpsimd.dma_start(out=outb_bc[:], in_=bcast_ap(outb_d[i], P))
            ff2b_bc = None
            if ff2b_nz:
                ff2b_bc = pools.tile([P, D], f32, tag="fbb", name="fbb")
                nc.gpsimd.dma_start(out=ff2b_bc[:], in_=bcast_ap(ff2b_d[i], P))
            ln1w_bc = ln1b_bc = ln2w_bc = ln2b_bc = None
            if ln1_nt:
                ln1w_bc = pools.tile([P, D], f32, tag="l1w", name="l1w")
                nc.gpsimd.dma_start(out=ln1w_bc[:], in_=bcast_ap(ln1w_d[i], P))
                ln1b_bc = pools.tile([P, D], f32, tag="l1b", name="l1b")
                nc.gpsimd.dma_start(out=ln1b_bc[:], in_=bcast_ap(ln1b_d[i], P))
            if ln2_nt:
                ln2w_bc = pools.tile([P, D], f32, tag="l2w", name="l2w")
                nc.gpsimd.dma_start(out=ln2w_bc[:], in_=bcast_ap(ln2w_d[i], P))
                ln2b_bc = pools.tile([P, D], f32, tag="l2b", name="l2b")
                nc.gpsimd.dma_start(out=ln2b_bc[:], in_=bcast_ap(ln2b_d[i], P))

            # preload the Exp activation table while the PE does transposes,
            # so the first attention exp doesn't stall on ACT_TABLE_LOAD
            dum = stat.tile([P, 1], f32, tag="dum", name="dum")
            nc.scalar.activation(dum[:], epst[:], AF.Exp, scale=1.0)

            xT = transpose_cast(x_t)

            # q,k feature-major [1536, 256]; emit q/k tile pairs interleaved
            # so attention head 0 (needs tiles 0 and 6) can start early
            qkT = [None] * NQK
            for gi, mo12 in enumerate(t for p in zip(range(KD), range(KD, NQK))
                                      for t in p):
                ps = psS.tile([P, P * MT], f32, tag="s", name="s")
                for ko in range(KD):
                    nc.tensor.matmul(ps[:], qkvw_t[ko][:, mo12 * P:(mo12 + 1) * P],
                                     xT[ko][:], start=(ko == 0), stop=(ko == KD - 1))
                t = qkTp.tile([P, P * MT], bf16, tag="qkT", name="qkT")
                if qkb_nz:
                    if gi % 2 == 0:
                        nc.scalar.activation(t[:], ps[:], AF.Identity,
                                             bias=qkvb_sb[:, i, mo12:mo12 + 1],
                                             scale=1.0)
                    else:
                        nc.vector.tensor_scalar_add(t[:], ps[:],
                                                    qkvb_sb[:, i, mo12:mo12 + 1])
                else:
                    nc.vector.tensor_copy(t[:], ps[:])
                qkT[mo12] = t

            # v token-major [256, 768] with interleaved ones columns per head:
            # v_aug [128, 12, 65] (col 64 of each head = 1.0)
            v_aug = [vp.tile([P, H, HD + 1], bf16, tag="vaug", name="vaug")
                     for _ in range(MT)]
            ps_v = [psW.tile([P, D], f32, tag="w", name="w") for _ in range(MT)]
            for ko in range(KD):
                for mo in range(MT):
                    mm_splits(nc, ps_v[mo], xT[ko][:, mo * P:(mo + 1) * P],
                              qkvw_t[ko][:, 2 * D:3 * D],
                              ((0, 512), (512, D)),
                              ko == 0, ko == KD - 1)
            for mo in range(MT):
                nc.vector.memset(v_aug[mo][:, :, HD:HD + 1], 1.0)
                src = ps_v[mo][:].rearrange("p (h d) -> p h d", d=HD)
                if vb_nz:
                    vb3 = vb_bc[:].rearrange("p (h d) -> p h d", d=HD)
                    nc.vector.scalar_tensor_tensor(
                        v_aug[mo][:, :, 0:HD], src, 1.0, vb3,
                        op0=OP.mult, op1=OP.add)
                else:
                    nc.vector.tensor_copy(v_aug[mo][:, :, 0:HD], src)

            # attention per (batch, head): scores computed k-major
            # (lhsT=k, rhs=q) so exp feeds AV directly as lhsT; AV output
            # [q, 65] per head packs into 2 psum tiles per batch; softmax
            # normalization batched into 2 recip + 2 strided muls.
            ao_bf = [vp.tile([P, D], bf16, tag="ao", name="ao") for _ in range(MT)]
            for b in range(MT):
                ps_ao_a = psW.tile([P, D], f32, tag="w", name="w")
                ps_ao_b = psW.tile([P, D], f32, tag="w", name="w")
                pk_a = ps_ao_a[:, 0:7 * (HD + 1)].rearrange(
                    "p (h c) -> p h c", c=HD + 1)
                pk_b = ps_ao_b[:, 0:5 * (HD + 1)].rearrange(
                    "p (h c) -> p h c", c=HD + 1)
                # software-pipelined: AV(h) issues 2 heads behind scores(h)
                # so the PE never waits the scores->exp scalar latency
                LAG = 3
                exs = [None] * H
                for idx in range(H + LAG):
                    if idx < H:
                        h = idx
                        t_idx, row0 = h // 2, (h % 2) * HD
                        q_ap = qkT[t_idx][row0:row0 + HD, b * P:(b + 1) * P]
                        k_ap = qkT[KD + t_idx][row0:row0 + HD, b * P:(b + 1) * P]
                        pool_s = psT if h % 2 == 0 else psS
                        ps_s = pool_s.tile([P, P], f32, tag=("t" if h % 2 == 0
                                                             else "s"), name="s")
                        nc.tensor.matmul(ps_s[:], k_ap, q_ap, start=True, stop=True)
                        ex = ap4.tile([P, P], bf16, tag="abf", name="abf")
                        nc.scalar.activation(ex[:], ps_s[:], AF.Exp,
                                             scale=1.0 / np.sqrt(HD))
                        exs[h] = ex
                    if idx >= LAG:
                        h = idx - LAG
                        pk = pk_a[:, h, :] if h < 7 else pk_b[:, h - 7, :]
                        va = v_aug[b][:, h, :]
                        nc.tensor.matmul(pk, exs[h][:], va, start=True, stop=True)
                # batched softmax normalization
                rec = stat.tile([P, H], f32, tag="rec", name="rec")
                nc.vector.reciprocal(rec[:, 0:7], pk_a[:, :, HD])
                nc.vector.reciprocal(rec[:, 7:H], pk_b[:, :, HD])
                ao3a = ao_bf[b][:, 0:7 * HD].rearrange("p (h d) -> p h d", d=HD)
                ao3b = ao_bf[b][:, 7 * HD:D].rearrange("p (h d) -> p h d", d=HD)
                nc.vector.tensor_mul(ao3a, pk_a[:, :, 0:HD],
                                     free_bcast(rec[:, 0:7], HD))
                nc.vector.tensor_mul(ao3b, pk_b[:, :, 0:HD],
                                     free_bcast(rec[:, 7:H], HD))

            # transpose ao to feature-major for the out-projection
            aoT = [aoTp.tile([P, P * MT], bf16, tag="aoT", name="aoT") for _ in range(KD)]
            for mo in range(MT):
                for ko in range(KD):
                    ps = psT.tile([P, P], bf16, tag="t", name="t")
                    nc.tensor.transpose(ps[:], ao_bf[mo][:, ko * P:(ko + 1) * P],
                                        idb[:])
                    nc.vector.tensor_copy(aoT[ko][:, mo * P:(mo + 1) * P], ps[:])

            # out-proj + residual + LN1 + re-transpose, mo-outer so batch 0's
            # vector-side LN overlaps batch 1's matmuls
            x1n_t = [None] * MT
            x1nT = [xTp.tile([P, P * MT], bf16, tag="xT", name="xT")
                    for _ in range(KD)]
            for mo in range(MT):
                ps_o = psW.tile([P, D], f32, tag="w", name="w")
                for ko in range(KD):
                    mm_splits(nc, ps_o, aoT[ko][:, mo * P:(mo + 1) * P],
                              outw_t[ko][:], ((0, 512), (512, D)),
                              ko == 0, ko == KD - 1)
                x1 = xpool.tile([P, D], f32, tag="x", name="x")
                nc.vector.scalar_tensor_tensor(
                    x1[:], ps_o[:], 1.0, x_t[mo][:],
                    op0=OP.mult, op1=OP.add)
                if outb_nz:
                    nc.vector.tensor_add(x1[:], x1[:], outb_bc[:])
                x1n_t[mo] = layernorm1(x1, ln1w_bc, ln1b_bc)
            for mo in range(MT):
                for ko in range(KD):
                    ps = psT.tile([P, P], f32, tag="t", name="t")
                    nc.tensor.transpose(ps[:], x1n_t[mo][:, ko * P:(ko + 1) * P],
                                        idf[:])
                    nc.vector.tensor_copy(x1nT[ko][:, mo * P:(mo + 1) * P], ps[:])

            # ff1 (relu) feature-major [3072, 256]
            hT = [hTp.tile([P, P * MT], bf16, tag="hT", name="hT") for _ in range(KF)]
            for mo24 in range(KF):
                ps = psS.tile([P, P * MT], f32, tag="s", name="s")
                for ko in range(KD):
                    nc.tensor.matmul(ps[:], ff1w_t[ko][:, mo24 * P:(mo24 + 1) * P],
                                     x1nT[ko][:], start=(ko == 0), stop=(ko == KD - 1))
                if f1b_nz:
                    if mo24 % 2 == 0:
                        nc.scalar.activation(hT[mo24][:], ps[:], AF.Relu,
                                             bias=ff1b_sb[:, i, mo24:mo24 + 1],
                                             scale=1.0)
                    else:
                        nc.vector.tensor_scalar(hT[mo24][:], ps[:],
                                                ff1b_sb[:, i, mo24:mo24 + 1], 0.0,
                                                op0=OP.add, op1=OP.max)
                else:
                    if mo24 % 2 == 0:
                        nc.scalar.activation(hT[mo24][:], ps[:], AF.Relu,
                                             scale=1.0)
                    else:
                        nc.vector.tensor_scalar_max(hT[mo24][:], ps[:], 0.0)

            # ff2 + residual + LN2. ko-outer so each ff2w tile's last read is
            # immediate and the 8-slot ring streams (mo-outer would pin all 24
            # tiles live and serialize the weight DMAs against the matmuls).
            x_t = [None] * MT
            ps_y = [psW.tile([P, D], f32, tag="w", name="w") for _ in range(MT)]
            KH = KF // 2
            for ko in range(KH):
                for mo in range(MT):
                    mm_splits(nc, ps_y[mo], hT[ko][:, mo * P:(mo + 1) * P],
                              ff2w_t[ko][:], ((0, 512), (512, D)),
                              ko == 0, False)
            # finish mo=0 first so its residual+LN overlaps mo=1's matmuls
            for mo in range(MT):
                for ko in range(KH, KF):
                    mm_splits(nc, ps_y[mo], hT[ko][:, mo * P:(mo + 1) * P],
                              ff2w_t[ko][:], ((0, 512), (512, D)),
                              False, ko == KF - 1)
                x2 = xpool.tile([P, D], f32, tag="x", name="x")
                nc.vector.scalar_tensor_tensor(
                    x2[:], ps_y[mo][:], 1.0, x1n_t[mo][:],
                    op0=OP.mult, op1=OP.add)
                if ff2b_nz:
                    nc.vector.tensor_add(x2[:], x2[:], ff2b_bc[:])
                x_t[mo] = layernorm1(x2, ln2w_bc, ln2b_bc)

            # throwaway matmuls bridge the LN tail so the PE's activity
            # monitor doesn't re-throttle the clock across the layer boundary
            for _dk in range(8):
                ps_dum = psT.tile([P, P * MT], f32, tag="t", name="dumm")
                nc.tensor.matmul(ps_dum[:], idb[:], xT[0][:],
                                 start=True, stop=True)
            if debug:
                for b in range(BL):
                    nc.sync.dma_start(out=xdbg_d[i + 1, b], in_=x_t[b][:])

            # spread classifier w2 prefetch over layers 1..3 to smooth the
            # HBM demand (classifier phase alone would oversubscribe DMA)
            if i >= 1:
                n0 = (i - 1) * 6
                n1 = i * 6 if i < NL - 1 else W2_PREFETCH
                for ci in range(n0, n1):
                    load_w2(ci)

        # ---- classifier ----
        xT = transpose_cast(x_t)
        ps_h = psS.tile([P, P * MT], f32, tag="s", name="s")
        for ko in range(KD):
            nc.tensor.matmul(ps_h[0:100, :], w1T_sb[:, ko, :], xT[ko][:],
                             start=(ko == 0), stop=(ko == KD - 1))
        hTa = const.tile([P, P * MT], bf16, tag="hTa", name="hTa")
        nc.vector.memset(hTa[:, :], 1.0)
        nc.vector.tensor_copy(hTa[0:100, :], ps_h[0:100, :])

        # logits: f32 psum halves (one bank each), bf16 evictions into a
        # 1024-wide bf16 tile, chunk-major output. psum slots alternate
        # between psS and psW; evictions split Scalar/Vector; out DMAs
        # alternate Sync/Scalar queues.
        HC = CCH // 2
        cseq = ((psS, "s"), (psT, "t"), (psW, "w"))
        cj = 0
        for ci in range(NCH):
            load_w2(ci + W2_PREFETCH)
            w2t = w2_tiles.pop(ci)
            for mo in range(MT):
                p0, tg0 = cseq[cj % 3]
                p1, tg1 = cseq[(cj + 1) % 3]
                cj += 2
                ps0 = p0.tile([P, HC], f32, tag=tg0, name="s")
                ps1 = p1.tile([P, HC], f32, tag=tg1, name="s")
                lhs = hTa[:, mo * P:(mo + 1) * P]
                nc.tensor.matmul(ps0[:], lhs, w2t[:, 0:HC], start=True, stop=True)
                mi = nc.tensor.matmul(ps1[:], lhs, w2t[:, HC:CCH],
                                      start=True, stop=True)
                mi.ins.ldweights = False
                ost = ostp.tile([P, CCH], bf16, tag="ost", name="ost")
                nc.scalar.copy(ost[:, 0:HC], ps0[:])
                nc.vector.tensor_copy(ost[:, HC:CCH], ps1[:])
                eng = nc.sync if mo == 0 else nc.gpsimd
                eng.dma_start(out=out_d[ci, mo], in_=ost[:])

    nc.compile()
    return nc


def _chunk_w2(cls_w2, cls_b2):
    # rows: 100 weights + 1 bias + 27 zero pad (lhsT rows 101.. are 1.0 from
    # the hTa memset, so the zero rows contribute nothing)
    w2a = np.concatenate(
        [cls_w2.T, cls_b2[None, :], np.zeros((27, NE), np.float32)], axis=0
    ).astype(BF16)  # [128, NE]
    pad = NCH * CCH - NE
    if pad:
        w2a = np.concatenate([w2a, np.zeros((128, pad), BF16)], axis=1)
    return np.ascontiguousarray(w2a.reshape(128, NCH, CCH).transpose(1, 0, 2))


def _prep(inputs):
    lhs = np.asarray(inputs["last_hidden_state"], dtype=np.float32)
    pos = np.asarray(inputs["entity_position_ids"])
    msk = np.asarray(inputs["entity_attention_mask"])
    qkv_w = np.asarray(inputs["qkv_w"], dtype=np.float32)
    qkv_b = np.asarray(inputs["qkv_b"], dtype=np.float32)
    out_w = np.asarray(inputs["out_w"], dtype=np.float32)
    out_b = np.asarray(inputs["out_b"], dtype=np.float32)
    ln1_w = np.asarray(inputs["ln1_w"], dtype=np.float32)
    ln1_b = np.asarray(inputs["ln1_b"], dtype=np.float32)
    ff1_w = np.asarray(inputs["ff1_w"], dtype=np.float32)
    ff1_b = np.asarray(inputs["ff1_b"], dtype=np.float32)
    ff2_w = np.asarray(inputs["ff2_w"], dtype=np.float32)
    ff2_b = np.asarray(inputs["ff2_b"], dtype=np.float32)
    ln2_w = np.asarray(inputs["ln2_w"], dtype=np.float32)
    ln2_b = np.asarray(inputs["ln2_b"], dtype=np.float32)
    cls_w1 = np.asarray(inputs["cls_w1"], dtype=np.float32)
    cls_w2 = np.asarray(inputs["cls_w2"], dtype=np.float32)
    cls_b2 = np.asarray(inputs["cls_b2"], dtype=np.float32)
    attn_w = np.asarray(inputs["attn_w"], dtype=np.float32)
    attn_b = float(np.asarray(inputs["attn_b"], dtype=np.float32))

    # ragged valid mask: 1 up to the first -1 (and only where attention mask set)
    nb = np.cumprod((pos != -1).astype(np.int32), axis=-1)
    valid = (msk != 0).astype(np.int32)[:, :, None] * nb       # [B, M, L]
    vmT = np.ascontiguousarray(valid.transpose(0, 2, 1)).astype(np.float32)

    cfg = (
        attn_b,
        bool(np.any(qkv_b[:, :2 * D])),
        bool(np.any(qkv_b[:, 2 * D:])),
        bool(np.any(out_b)),
        bool(np.any(ff1_b)),
        bool(np.any(ff2_b)),
        not (np.all(ln1_w == 1.0) and np.all(ln1_b == 0.0)),
        not (np.all(ln2_w == 1.0) and np.all(ln2_b == 0.0)),
        bool(KERNEL_DEBUG),
    )

    shared = {
        "attnw": attn_w,
        "qkvw": np.ascontiguousarray(qkv_w.transpose(0, 2, 1)).reshape(
            NL, KD, P, 3 * D).astype(BF16),
        "outw": np.ascontiguousarray(out_w.transpose(0, 2, 1)).reshape(
            NL, KD, P, D).astype(BF16),
        "ff1w": np.ascontiguousarray(ff1_w.transpose(0, 2, 1)).reshape(
            NL, KD, P, DFF).astype(BF16),
        "ff2w": np.ascontiguousarray(ff2_w.transpose(0, 2, 1)).reshape(
            NL, KF, P, D).astype(BF16),
        "w1T": np.ascontiguousarray(cls_w1.T).reshape(KD, P, 100).astype(BF16),
        "w2a": _chunk_w2(cls_w2, cls_b2),
    }
    if cfg[1] or cfg[2]:
        shared["qkvb"] = qkv_b
    if cfg[3]:
        shared["outb"] = out_b
    if cfg[4]:
        shared["ff1b"] = ff1_b
    if cfg[5]:
        shared["ff2b"] = ff2_b
    if cfg[6]:
        shared["ln1w"] = ln1_w
        shared["ln1b"] = ln1_b
    if cfg[7]:
        shared["ln2w"] = ln2_w
        shared["ln2b"] = ln2_b

    lhs32 = np.ascontiguousarray(lhs[:, :L, :])
    in_maps = []
    for c in range(N_CORES):
        m = dict(shared)
        m["lhs32"] = np.ascontiguousarray(lhs32[c * BL:(c + 1) * BL])
        m["vmT"] = np.ascontiguousarray(vmT[c * BL:(c + 1) * BL])
        in_maps.append(m)
    return cfg, in_maps


def kernel(**inputs):
    from concourse.bass_utils import run_bass_kernel_spmd

    cfg, in_maps = _prep(inputs)
    if cfg not in _CACHE:
        _CACHE[cfg] = _build(cfg)
    nc = _CACHE[cfg]
    res = run_bass_kernel_spmd(nc, in_maps, core_ids=list(range(N_CORES)))
    parts = []
    for c in range(N_CORES):
        o = res.results[c]["out2"]  # [NCH, MT, P, CCH] bf16
        o = np.ascontiguousarray(o.transpose(1, 2, 0, 3)).reshape(BL, M, NCH * CCH)
        parts.append(o[:, :, :NE].astype(np.float32))
    out = np.concatenate(parts, axis=0)
    if KERNEL_DEBUG:
        kernel.last_debug = [res.results[c].get("xdbg") for c in range(N_CORES)]
    return out
